# revision 9
# baseline (speedup 1.0000x reference)
"""CapsuleConv2d Trainium2 kernel.

Math: out[b,o,h,w,i,j] = sum_{ci,kh,kw} W[j,o,ci,kh,kw] * x[b,ci,h+kh-1,w+kw-1,i,0]
i.e. a 3x3 pad-1 conv with effective batch (b,i): 64 images [64,56,56],
Cout = 256 (co = j*64+o).

Strategy (8 cores, data-parallel over b):
  - each core takes 2 of 16 b-groups; the 4 ic0 images of a b-group ride in
    the free dim (w,i) so HBM loads are fully contiguous.
  - x lives in SBUF as [ci, h_pad, (w_pad, i)] with a zero halo; each conv
    offset (kh,kw) is a sliced matmul rhs, accumulated in PSUM over 9 offsets.
  - Cin=64 only fills half the 128-row PE array: partitions 64..127 hold a
    duplicate of x, and the co-high half's matmuls run there concurrently
    (row tiling via base_partition) -> 2x PE throughput.
  - float32r matmuls (full rate at N=448), fp32 PSUM accumulate, fp32 output.
  - host does the final (b,o,h,w,i,j) layout transpose (not device time).
"""

import sys

if "/opt/trn_rl_repo" not in sys.path:
    sys.path.insert(0, "/opt/trn_rl_repo")

import numpy as np

NCORES = 8
B, C, H, W_, IC0, WC1, O = 16, 64, 56, 56, 4, 4, 64
CO = WC1 * O  # 256
BPC = B // NCORES  # b-groups per core
WI = W_ * IC0  # 224 = free-dim run per image row
HP, WP = H + 2, (W_ + 2) * IC0  # padded sbuf tile dims: 58, 232
NKER = 9

_COMPILED = None
_W16 = False


def _build(reps=1, mode='full', out_bf16=False, nchunks=4, w16=False):
    import concourse.tile as tile
    from concourse import bacc, mybir

    dt = mybir.dt
    DT = dt.float32r

    nc = bacc.Bacc(
        "TRN2", target_bir_lowering=False, debug=False, num_devices=NCORES
    )
    wdt = dt.float16 if w16 else DT
    x_d = nc.dram_tensor("x", [BPC, C, HP, WP], DT, kind="ExternalInput").ap()
    w_d = nc.dram_tensor("w", [128, NKER, 128], wdt, kind="ExternalInput").ap()
    ydt = dt.bfloat16 if out_bf16 else dt.float32
    y_d = nc.dram_tensor(
        "y", [BPC, CO, H, WI], ydt, kind="ExternalOutput"
    ).ap()

    with tile.TileContext(nc) as tc:
        with (
            tc.tile_pool(name="xp", bufs=1) as xp,
            tc.tile_pool(name="wp", bufs=1) as wp,
            tc.tile_pool(name="op", bufs=3) as op,
            tc.tile_pool(name="pp", bufs=2, space="PSUM") as pp,
        ):
            wt = wp.tile([128, NKER, 128], wdt)
            nc.sync.dma_start(wt[:, :, :], w_d[:, :, :])

            xts = [
                xp.tile([128, HP, WP], DT, tag=f"x{b}", name=f"x{b}")
                for b in range(BPC)
            ]
            for rep in range(reps):
              for b in range(BPC):  # noqa: E111
                xt = xts[b]
                bounds = [HP * i // nchunks for i in range(nchunks)] + [HP]
                for r0, r1 in zip(bounds, bounds[1:]):
                    nc.sync.dma_start(xt[0:64, r0:r1, :], x_d[b, :, r0:r1, :])
                    nc.sync.dma_start(xt[64:128, r0:r1, :], xt[0:64, r0:r1, :])

                for ht in range(H // 2):
                    h0 = 2 * ht
                    if mode == 'dma_only':
                        oA = op.tile([128, 2, WI], ydt, tag="oA")
                        oB = op.tile([128, 2, WI], ydt, tag="oB")
                        nc.vector.tensor_copy(
                            oA[:, :, :], xt[:, h0 : h0 + 2, 0:WI].bitcast(dt.float32))
                        nc.vector.tensor_copy(
                            oB[:, :, :], xt[:, h0 + 1 : h0 + 3, 0:WI].bitcast(dt.float32))
                        nc.sync.dma_start(y_d[b, 0:128, h0 : h0 + 2, :], oA[:, :, :])
                        nc.sync.dma_start(y_d[b, 128:256, h0 : h0 + 2, :], oB[:, :, :])
                        continue
                    pA = pp.tile([128, 2, WI], dt.float32, tag="pA")
                    pB = pp.tile([128, 2, WI], dt.float32, tag="pB")
                    for k in range(NKER):
                        kh, kw = divmod(k, 3)
                        c0 = IC0 * kw
                        nc.tensor.matmul(
                            pA[:, :, :],
                            lhsT=wt[0:64, k, :],
                            rhs=xt[0:64, h0 + kh : h0 + kh + 2, c0 : c0 + WI],
                            start=(k == 0),
                            stop=(k == NKER - 1),
                        )
                        nc.tensor.matmul(
                            pB[:, :, :],
                            lhsT=wt[64:128, k, :],
                            rhs=xt[64:128, h0 + kh : h0 + kh + 2, c0 : c0 + WI],
                            start=(k == 0),
                            stop=(k == NKER - 1),
                        )
                    if mode == 'pe_only':
                        continue
                    oA = op.tile([128, 2, WI], ydt, tag="oA")
                    oB = op.tile([128, 2, WI], ydt, tag="oB")
                    nc.vector.tensor_copy(oA[:, :, :], pA[:, :, :])
                    nc.vector.tensor_copy(oB[:, :, :], pB[:, :, :])
                    nc.sync.dma_start(y_d[b, 0:128, h0 : h0 + 2, :], oA[:, :, :])
                    nc.sync.dma_start(y_d[b, 128:256, h0 : h0 + 2, :], oB[:, :, :])

    nc.compile()
    return nc


def _prep(x, W):
    x = np.asarray(x, dtype=np.float32)
    W = np.asarray(W, dtype=np.float32)
    wnp = np.float16 if _W16 else np.float32
    xs = x.reshape(B, C, H, WI)  # drop ic1, fuse (w,i)
    xpad = np.zeros((B, C, HP, WP), np.float32)  # host zero-pad = sbuf halo
    xpad[:, :, 1 : H + 1, IC0 : IC0 + WI] = xs
    Wf = W.reshape(CO, C, 3, 3)
    Wt = np.ascontiguousarray(Wf.transpose(1, 2, 3, 0)).reshape(C, NKER, CO)
    wsb = np.ascontiguousarray(
        np.concatenate([Wt[:, :, :128], Wt[:, :, 128:]], axis=0).astype(wnp)
    )  # [128, 9, 128]; rows 0-63 ci for co-low, 64-127 ci for co-high
    return xpad, wsb


def _run(x, W, trace=False):
    global _COMPILED
    from concourse.bass_utils import run_bass_kernel_spmd

    if _COMPILED is None:
        _COMPILED = _build()
    nc = _COMPILED
    xs, wsb = _prep(x, W)
    in_maps = [
        {"x": np.ascontiguousarray(xs[c * BPC : (c + 1) * BPC]), "w": wsb}
        for c in range(NCORES)
    ]
    res = run_bass_kernel_spmd(
        nc, in_maps, core_ids=list(range(NCORES)), trace=trace
    )
    ys = np.concatenate(
        [np.asarray(res.results[c]["y"], dtype=np.float32) for c in range(NCORES)],
        axis=0,
    )  # [16, 256, 56, 224]
    out = (
        ys.reshape(B, WC1, O, H, W_, IC0)
        .transpose(0, 2, 3, 4, 5, 1)
        .astype(np.float32)
    )
    return np.ascontiguousarray(out), res


def kernel(**inputs) -> np.ndarray:
    return _run(inputs["x"], inputs["W"])[0]
